# revision 27
# baseline (speedup 1.0000x reference)
"""AnomalyAttention forward (causal attention returning both V and the full
softmax matrix) as a distributed Bass kernel on 8 TRN2 NeuronCores.

Sharding: the 32 (batch, head) pairs are split 4-per-core (data+head
parallel, no cross-core comms). Per pair the kernel does, on-device:

  pass 1 ([s, l] orientation):  S^T = K @ Q^T on TensorE (bf16, contraction
      E=64), additive causal mask on diagonal blocks, exp on ScalarE, then
      PV matmuls with an appended ones-column so PSUM accumulates
      [sum_s exp*V | sum_s exp]  ->  attention numerator + softmax denom.
  pass 2 ([l, s] orientation):  recompute scores transposed, exp on ScalarE,
      normalize rows by 1/denom (per-partition scalar on GPSIMD), emitting
      softmax rows directly in the output layout.  Only the causal
      (lower-triangular) region is written; output DRAM is pre-zeroed.

The next pair's load + Q/K transposes are emitted mid-pair so the TensorE
stream never blocks ACT at pair boundaries.
"""

import numpy as np

import concourse.bass as bass
import concourse.mybir as mybir
import concourse.tile as tile
from concourse import bacc
from concourse.bass_utils import run_bass_kernel_spmd

B, L, H, E = 4, 2048, 8, 64
D = 64
NCORES = 8
PAIRS = B * H            # 32
PPC = PAIRS // NCORES    # pairs per core = 4
NT = L // 512            # 4 l_tiles of 512
F32 = mybir.dt.float32
BF16 = mybir.dt.bfloat16
AF = mybir.ActivationFunctionType
ALU = mybir.AluOpType
SCALE = 1.0 / 8.0        # 1/sqrt(E)
NEG = -1.0e9             # additive mask value (exp(NEG*SCALE) == 0)

_CACHE = {}
_SKIP_SERIES_DMA = False
_SKIP_PASS2 = False
_SERIES_DMA_SPLIT = 2   # 0: SP only; 1: SP/ACT alternate; 2: SP/ACT/GPSIMD
_SCALE_ON_DVE = True
_SERIES_SPLIT_HALVES = False
_NO_ACT_RING = True
_PS2_BUFS = 2
_PV_SINGLE_BANK = False


def _build_nc(loop_n=None):
    """loop_n: if set, wrap the whole body in an on-device For_i loop
    (used only for wall-clock benchmarking — amortizes RPC latency)."""
    nc = bacc.Bacc("TRN2", target_bir_lowering=False, debug=False,
                   num_devices=NCORES)
    q_ext = nc.dram_tensor("q", [PPC, L, E], F32, kind="ExternalInput")
    k_ext = nc.dram_tensor("k", [PPC, L, E], F32, kind="ExternalInput")
    v_ext = nc.dram_tensor("v", [PPC, L, D], F32, kind="ExternalInput")
    vout_ext = nc.dram_tensor("vout", [PPC, L, D], F32, kind="ExternalOutput")
    ser_ext = nc.dram_tensor("series", [PPC, L, L], F32, kind="ExternalOutput")

    with tile.TileContext(nc) as tc:
        with (
            tc.tile_pool(name="const", bufs=1) as constp,
            tc.tile_pool(name="qk", bufs=2) as qkp,
            tc.tile_pool(name="stage", bufs=2) as stagep,
            tc.tile_pool(name="x", bufs=4) as xp,
            tc.tile_pool(name="row", bufs=4) as rowp,
            tc.tile_pool(name="small", bufs=2) as smallp,
            # PSUM budget (8 banks of 2KB): ps1 2x2 banks, ps2 2x1, pv 2x1.
            # NB: a start=True matmul zeroes its whole 2KB bank; regions that
            # share a bank use start=True only on the bank's first writer
            # (the zero-region also clears has_written for the whole bank, so
            # later start=False writers overwrite, then accumulate).
            tc.tile_pool(name="ps1", bufs=2, space="PSUM") as ps1,
            tc.tile_pool(name="ps2", bufs=_PS2_BUFS, space="PSUM") as ps2,
            tc.tile_pool(name="pspv", bufs=1, space="PSUM") as pspv,
        ):
            # ---- constants -------------------------------------------------
            ident = constp.tile([128, 128], F32)
            nc.gpsimd.memset(ident[:], 0.0)
            nc.gpsimd.affine_select(
                out=ident[:], in_=ident[:], compare_op=ALU.not_equal,
                fill=1.0, base=0, pattern=[[-1, 128]], channel_multiplier=1,
            )
            # pass-1 diagonal masks (orientation [s, l]): slot covers l_rel in
            # [0,512) of an l_tile, partition is s_rel of block j where
            # 128*j - 512*t = 128*m.  keep iff  l_rel >= s_rel + 128*m.
            mask512 = constp.tile([128, 4, 512], F32)
            nc.gpsimd.memset(mask512[:], 0.0)
            for m in range(4):
                nc.gpsimd.affine_select(
                    out=mask512[:, m, :], in_=mask512[:, m, :],
                    compare_op=ALU.is_ge, fill=NEG, base=-128 * m,
                    pattern=[[1, 512]], channel_multiplier=-1,
                )
            # pass-2 diagonal mask (orientation [l, s]): keep iff s_rel <= l_rel
            mask_d2 = constp.tile([128, 128], F32)
            nc.gpsimd.memset(mask_d2[:], 0.0)
            nc.gpsimd.affine_select(
                out=mask_d2[:], in_=mask_d2[:], compare_op=ALU.is_ge,
                fill=NEG, base=0, pattern=[[-1, 128]], channel_multiplier=1,
            )

            def load_pair(p):
                qn = stagep.tile([128, 16, E], F32, tag="qn")
                kn = stagep.tile([128, 16, E], F32, tag="kn")
                vn = stagep.tile([128, 16, D], F32, tag="vn")
                va = stagep.tile([128, 16, D + 1], BF16, tag="va")
                nc.sync.dma_start(
                    qn[:], q_ext[p].rearrange("(n pp) e -> pp n e", pp=128))
                (nc.sync if _NO_ACT_RING else nc.scalar).dma_start(
                    kn[:], k_ext[p].rearrange("(n pp) e -> pp n e", pp=128))
                nc.gpsimd.dma_start(
                    vn[:], v_ext[p].rearrange("(n pp) d -> pp n d", pp=128))
                nc.vector.tensor_copy(va[:, :, 0:D], vn[:])
                nc.vector.memset(va[:, :, D], 1.0)
                return qn, kn, va

            def transpose_pair(qn, kn):
                qt = qkp.tile([E, L], BF16, tag="qt")
                kt = qkp.tile([E, L], BF16, tag="kt")
                for g4 in range(4):
                    for si, (src, dst) in enumerate(((qn, qt), (kn, kt))):
                        # 4 transposes share one bank: first-touch start=True.
                        # Alternate psum tags for a 4-deep rotation.
                        pool_tag = "s1" if (2 * g4 + si) % 2 == 0 else "s2"
                        pool = ps1 if pool_tag == "s1" else ps2
                        pst = pool.tile([E, 4, 128], F32, tag=pool_tag)
                        for u in range(4):
                            n = 4 * g4 + u
                            nc.tensor.matmul(
                                pst[:, u, :], lhsT=src[:, n, :], rhs=ident[:],
                                is_transpose=True, start=(u == 0),
                                stop=(u == 3), skip_group_check=True)
                        nc.vector.tensor_copy(
                            dst[:, g4 * 512:(g4 + 1) * 512].rearrange(
                                "p (u f) -> p u f", u=4),
                            pst[:])
                return qt, kt

            def pass1_tile(t, qt, kt, va):
                # 4 chunk accumulators share one bank; banks alternate
                # across tiles so tile t+1's first (start=True) PV matmul
                # never waits on tile t's PSUM readers.
                pvps = pspv.tile([128, 4, 128], F32,
                 tag="pv0" if _PV_SINGLE_BANK else f"pv{t % 2}")
                for g in range(2 * (t + 1)):
                    pss = ps1.tile([128, 1024], F32, tag="s1")
                    for jj in range(2):
                        j = 2 * g + jj
                        nc.tensor.matmul(
                            pss[:, jj * 512:(jj + 1) * 512],
                            lhsT=kt[:, j * 128:(j + 1) * 128],
                            rhs=qt[:, t * 512:(t + 1) * 512],
                            start=True, stop=True)
                    for jj in range(2):
                        m = 2 * g + jj - 4 * t
                        if 0 <= m < 4:
                            nc.vector.tensor_add(
                                pss[:, jj * 512:(jj + 1) * 512],
                                pss[:, jj * 512:(jj + 1) * 512],
                                mask512[:, m, :])
                    xg = xp.tile([128, 1024], BF16, tag="x")
                    nc.scalar.activation(xg[:], pss[:], AF.Exp, scale=SCALE)
                    for jj in range(2):
                        j = 2 * g + jj
                        for c in range(4):
                            C = 4 * t + c
                            if j <= C:
                                nc.tensor.matmul(
                                    pvps[:, c, 0:D + 1],
                                    lhsT=xg[:, jj * 512 + c * 128:
                                            jj * 512 + c * 128 + 128],
                                    rhs=va[:, j, :],
                                    start=(j == 0 and c == 0),
                                    stop=(j == C),
                                    skip_group_check=True)
                return pvps

            def denoms_and_vout(p, t, pvps):
                rcp = smallp.tile([128, 4], F32, tag="rcp")
                nc.vector.reciprocal(rcp[:], pvps[:, :, D])
                vsb = smallp.tile([128, 4, D], F32, tag="vsb")
                for c in range(4):
                    nc.vector.tensor_scalar_mul(
                        vsb[:, c, :], pvps[:, c, 0:D], rcp[:, c:c + 1])
                (nc.sync if (t % 2 == 0 or _NO_ACT_RING)
                 else nc.scalar).dma_start(
                    vout_ext[p, t * 512:(t + 1) * 512, :].rearrange(
                        "(c pp) d -> pp c d", pp=128),
                    vsb[:])
                return rcp

            _rings = (nc.sync, nc.scalar, nc.gpsimd)

            def series_dma(idx, dst, srcap):
                if _SKIP_SERIES_DMA:
                    return
                if _SERIES_DMA_SPLIT == 0:
                    nc.sync.dma_start(dst, srcap)
                elif _SERIES_DMA_SPLIT == 1:
                    (nc.sync, nc.scalar)[idx % 2].dma_start(dst, srcap)
                elif _NO_ACT_RING:
                    (nc.sync, nc.gpsimd)[idx % 2].dma_start(dst, srcap)
                elif not _SERIES_SPLIT_HALVES:
                    _rings[idx % 3].dma_start(dst, srcap)
                else:
                    _rings[(2 * idx) % 3].dma_start(dst[0:64], srcap[0:64])
                    _rings[(2 * idx + 1) % 3].dma_start(
                        dst[64:128], srcap[64:128])

            def pass2_tile(p, t, qt, kt, rcp, is_tail):
                if _SKIP_PASS2:
                    return
                for c in range(4):
                    C = 4 * t + c
                    W = (C + 1) * 128
                    row = rowp.tile([128, 2048], F32, tag="row")
                    tail_chunk = is_tail and c == 3
                    base = 0
                    while base < W:
                        w = min(512, W - base)
                        psr = ps2.tile([128, 512], F32, tag="s2")
                        nc.tensor.matmul(
                            psr[:, 0:w],
                            lhsT=qt[:, C * 128:(C + 1) * 128],
                            rhs=kt[:, base:base + w],
                            start=True, stop=True)
                        if base + w == W:
                            nc.vector.tensor_add(
                                psr[:, w - 128:w], psr[:, w - 128:w],
                                mask_d2[:])
                        nc.scalar.activation(
                            row[:, base:base + w], psr[:, 0:w],
                            AF.Exp, scale=SCALE)
                        if tail_chunk:
                            # kernel tail: normalize + store per instalment on
                            # DVE so the last DMA starts as early as possible
                            nc.vector.tensor_scalar_mul(
                                row[:, base:base + w], row[:, base:base + w],
                                rcp[:, c:c + 1])
                            series_dma(
                                base // 512,
                                ser_ext[p, C * 128:(C + 1) * 128,
                                        base:base + w],
                                row[:, base:base + w])
                        base += w
                    if not tail_chunk:
                        # normalize rows by 1/denom
                        eng = nc.vector if _SCALE_ON_DVE else nc.gpsimd
                        eng.tensor_scalar_mul(
                            row[:, 0:W], row[:, 0:W], rcp[:, c:c + 1])
                        series_dma(
                            4 * t + c,
                            ser_ext[p, C * 128:(C + 1) * 128, 0:W],
                            row[:, 0:W])

            # ---- software pipeline over the 16 (pair, tile) units:
            # pass-1 of unit i+1 is emitted before pass-2 of unit i, so the
            # in-order ACT stream always has a segment whose inputs are ready.
            def emit_body():
                units = [(p, t) for p in range(PPC) for t in range(NT)]
                pair_bufs = {}
                qn0, kn0, va0 = load_pair(0)
                qt0, kt0 = transpose_pair(qn0, kn0)
                pair_bufs[0] = (qt0, kt0, va0)

                def unit_pass1(i):
                    p, t = units[i]
                    qt, kt, va = pair_bufs[p]
                    pv = pass1_tile(t, qt, kt, va)
                    if t == 1 and p + 1 < PPC:
                        qn2, kn2, va2 = load_pair(p + 1)
                        qt2, kt2 = transpose_pair(qn2, kn2)
                        pair_bufs[p + 1] = (qt2, kt2, va2)
                    return pv

                pv = unit_pass1(0)
                for i, (p, t) in enumerate(units):
                    rcp = denoms_and_vout(p, t, pv)
                    if i + 1 < len(units):
                        pv = unit_pass1(i + 1)
                    qt, kt, _ = pair_bufs[p]
                    pass2_tile(p, t, qt, kt, rcp,
                               is_tail=(i == len(units) - 1))
                    if t == NT - 1:
                        del pair_bufs[p]

            if loop_n is None:
                emit_body()
            else:
                with tc.For_i(0, loop_n, 1):
                    emit_body()

    nc.finalize()
    return nc


def kernel(queries, keys, values, sigma=None, **_unused):
    if "nc" not in _CACHE:
        _CACHE["nc"] = _build_nc()
    nc = _CACHE["nc"]

    q = np.ascontiguousarray(
        np.asarray(queries, np.float32).transpose(0, 2, 1, 3).reshape(PAIRS, L, E))
    k = np.ascontiguousarray(
        np.asarray(keys, np.float32).transpose(0, 2, 1, 3).reshape(PAIRS, L, E))
    v = np.ascontiguousarray(
        np.asarray(values, np.float32).transpose(0, 2, 1, 3).reshape(PAIRS, L, D))

    in_maps = [
        {"q": q[i * PPC:(i + 1) * PPC],
         "k": k[i * PPC:(i + 1) * PPC],
         "v": v[i * PPC:(i + 1) * PPC]}
        for i in range(NCORES)
    ]
    res = run_bass_kernel_spmd(nc, in_maps, list(range(NCORES)))

    vout = np.concatenate([res.results[i]["vout"] for i in range(NCORES)], 0)
    vout = vout.reshape(B, H, L, D).transpose(0, 2, 1, 3)
    ser = np.concatenate([res.results[i]["series"] for i in range(NCORES)], 0)
    ser = ser.reshape(B, H, L, L)
    return np.ascontiguousarray(vout), ser


# revision 30
# speedup vs baseline: 4.3445x; 4.3445x over previous
"""AnomalyAttention forward (causal attention returning both V and the full
softmax matrix) as a distributed Bass kernel on 8 TRN2 NeuronCores.

Sharding: the 32 (batch, head) pairs are split 4-per-core (data+head
parallel, no cross-core comms). Per pair the kernel does, on-device:

  pass 1 ([s, l] orientation):  S^T = K @ Q^T on TensorE (bf16, contraction
      E=64), additive causal mask on diagonal blocks, exp on ScalarE, then
      PV matmuls with an appended ones-column so PSUM accumulates
      [sum_s exp*V | sum_s exp]  ->  attention numerator + softmax denom.
  pass 2 ([l, s] orientation):  recompute scores transposed, exp on ScalarE,
      normalize rows by 1/denom (per-partition scalar on GPSIMD), emitting
      softmax rows directly in the output layout.  Only the causal
      (lower-triangular) region is written; output DRAM is pre-zeroed.

The next pair's load + Q/K transposes are emitted mid-pair so the TensorE
stream never blocks ACT at pair boundaries.
"""

import numpy as np

import concourse.bass as bass
import concourse.mybir as mybir
import concourse.tile as tile
from concourse import bacc
from concourse.bass_utils import run_bass_kernel_spmd

B, L, H, E = 4, 2048, 8, 64
D = 64
NCORES = 8
PAIRS = B * H            # 32
PPC = PAIRS // NCORES    # pairs per core = 4
NT = L // 512            # 4 l_tiles of 512
F32 = mybir.dt.float32
BF16 = mybir.dt.bfloat16
AF = mybir.ActivationFunctionType
ALU = mybir.AluOpType
SCALE = 1.0 / 8.0        # 1/sqrt(E)
NEG = -1.0e9             # additive mask value (exp(NEG*SCALE) == 0)

_CACHE = {}
_SKIP_SERIES_DMA = False
_SKIP_PASS2 = False
_SERIES_DMA_SPLIT = 1   # 0: SP only; 1: SP/ACT alternate; 2: SP/ACT/GPSIMD
_SCALE_ON_DVE = True
_SERIES_SPLIT_HALVES = False
_NO_ACT_RING = False
_PS2_BUFS = 2
_PV_SINGLE_BANK = False


def _build_nc(loop_n=None):
    """loop_n: if set, wrap the whole body in an on-device For_i loop
    (used only for wall-clock benchmarking — amortizes RPC latency)."""
    nc = bacc.Bacc("TRN2", target_bir_lowering=False, debug=False,
                   num_devices=NCORES)
    q_ext = nc.dram_tensor("q", [PPC, L, E], F32, kind="ExternalInput")
    k_ext = nc.dram_tensor("k", [PPC, L, E], F32, kind="ExternalInput")
    v_ext = nc.dram_tensor("v", [PPC, L, D], F32, kind="ExternalInput")
    vout_ext = nc.dram_tensor("vout", [PPC, L, D], F32, kind="ExternalOutput")
    ser_ext = nc.dram_tensor("series", [PPC, L, L], F32, kind="ExternalOutput")

    with tile.TileContext(nc) as tc:
        with (
            tc.tile_pool(name="const", bufs=1) as constp,
            tc.tile_pool(name="qk", bufs=2) as qkp,
            tc.tile_pool(name="stage", bufs=2) as stagep,
            tc.tile_pool(name="x", bufs=4) as xp,
            tc.tile_pool(name="row", bufs=4) as rowp,
            tc.tile_pool(name="small", bufs=2) as smallp,
            # PSUM budget (8 banks of 2KB): ps1 2x2 banks, ps2 2x1, pv 2x1.
            # NB: a start=True matmul zeroes its whole 2KB bank; regions that
            # share a bank use start=True only on the bank's first writer
            # (the zero-region also clears has_written for the whole bank, so
            # later start=False writers overwrite, then accumulate).
            tc.tile_pool(name="ps1", bufs=2, space="PSUM") as ps1,
            tc.tile_pool(name="ps2", bufs=_PS2_BUFS, space="PSUM") as ps2,
            tc.tile_pool(name="pspv", bufs=1, space="PSUM") as pspv,
        ):
            # ---- constants -------------------------------------------------
            ident = constp.tile([128, 128], F32)
            nc.gpsimd.memset(ident[:], 0.0)
            nc.gpsimd.affine_select(
                out=ident[:], in_=ident[:], compare_op=ALU.not_equal,
                fill=1.0, base=0, pattern=[[-1, 128]], channel_multiplier=1,
            )
            # pass-1 diagonal masks (orientation [s, l]): slot covers l_rel in
            # [0,512) of an l_tile, partition is s_rel of block j where
            # 128*j - 512*t = 128*m.  keep iff  l_rel >= s_rel + 128*m.
            mask512 = constp.tile([128, 4, 512], F32)
            nc.gpsimd.memset(mask512[:], 0.0)
            for m in range(4):
                nc.gpsimd.affine_select(
                    out=mask512[:, m, :], in_=mask512[:, m, :],
                    compare_op=ALU.is_ge, fill=NEG, base=-128 * m,
                    pattern=[[1, 512]], channel_multiplier=-1,
                )
            # pass-2 diagonal mask (orientation [l, s]): keep iff s_rel <= l_rel
            mask_d2 = constp.tile([128, 128], F32)
            nc.gpsimd.memset(mask_d2[:], 0.0)
            nc.gpsimd.affine_select(
                out=mask_d2[:], in_=mask_d2[:], compare_op=ALU.is_ge,
                fill=NEG, base=0, pattern=[[-1, 128]], channel_multiplier=1,
            )

            def load_pair(p):
                qn = stagep.tile([128, 16, E], F32, tag="qn")
                kn = stagep.tile([128, 16, E], F32, tag="kn")
                vn = stagep.tile([128, 16, D], F32, tag="vn")
                va = stagep.tile([128, 16, D + 1], BF16, tag="va")
                nc.sync.dma_start(
                    qn[:], q_ext[p].rearrange("(n pp) e -> pp n e", pp=128))
                (nc.sync if _NO_ACT_RING else nc.scalar).dma_start(
                    kn[:], k_ext[p].rearrange("(n pp) e -> pp n e", pp=128))
                nc.gpsimd.dma_start(
                    vn[:], v_ext[p].rearrange("(n pp) d -> pp n d", pp=128))
                nc.vector.tensor_copy(va[:, :, 0:D], vn[:])
                nc.vector.memset(va[:, :, D], 1.0)
                return qn, kn, va

            def transpose_pair(qn, kn):
                qt = qkp.tile([E, L], BF16, tag="qt")
                kt = qkp.tile([E, L], BF16, tag="kt")
                for g4 in range(4):
                    for si, (src, dst) in enumerate(((qn, qt), (kn, kt))):
                        # 4 transposes share one bank: first-touch start=True.
                        # Alternate psum tags for a 4-deep rotation.
                        pool_tag = "s1" if (2 * g4 + si) % 2 == 0 else "s2"
                        pool = ps1 if pool_tag == "s1" else ps2
                        pst = pool.tile([E, 4, 128], F32, tag=pool_tag)
                        for u in range(4):
                            n = 4 * g4 + u
                            nc.tensor.matmul(
                                pst[:, u, :], lhsT=src[:, n, :], rhs=ident[:],
                                is_transpose=True, start=(u == 0),
                                stop=(u == 3), skip_group_check=True)
                        nc.vector.tensor_copy(
                            dst[:, g4 * 512:(g4 + 1) * 512].rearrange(
                                "p (u f) -> p u f", u=4),
                            pst[:])
                return qt, kt

            def pass1_tile(t, qt, kt, va):
                # 4 chunk accumulators share one bank; banks alternate
                # across tiles so tile t+1's first (start=True) PV matmul
                # never waits on tile t's PSUM readers.
                pvps = pspv.tile([128, 4, 128], F32,
                 tag="pv0" if _PV_SINGLE_BANK else f"pv{t % 2}")
                for g in range(2 * (t + 1)):
                    pss = ps1.tile([128, 1024], F32, tag="s1")
                    for jj in range(2):
                        j = 2 * g + jj
                        nc.tensor.matmul(
                            pss[:, jj * 512:(jj + 1) * 512],
                            lhsT=kt[:, j * 128:(j + 1) * 128],
                            rhs=qt[:, t * 512:(t + 1) * 512],
                            start=True, stop=True)
                    for jj in range(2):
                        m = 2 * g + jj - 4 * t
                        if 0 <= m < 4:
                            nc.vector.tensor_add(
                                pss[:, jj * 512:(jj + 1) * 512],
                                pss[:, jj * 512:(jj + 1) * 512],
                                mask512[:, m, :])
                    xg = xp.tile([128, 1024], BF16, tag="x")
                    nc.scalar.activation(xg[:], pss[:], AF.Exp, scale=SCALE)
                    for jj in range(2):
                        j = 2 * g + jj
                        for c in range(4):
                            C = 4 * t + c
                            if j <= C:
                                nc.tensor.matmul(
                                    pvps[:, c, 0:D + 1],
                                    lhsT=xg[:, jj * 512 + c * 128:
                                            jj * 512 + c * 128 + 128],
                                    rhs=va[:, j, :],
                                    start=(j == 0 and c == 0),
                                    stop=(j == C),
                                    skip_group_check=True)
                return pvps

            def denoms_and_vout(p, t, pvps):
                rcp = smallp.tile([128, 4], F32, tag="rcp")
                nc.vector.reciprocal(rcp[:], pvps[:, :, D])
                vsb = smallp.tile([128, 4, D], F32, tag="vsb")
                for c in range(4):
                    nc.vector.tensor_scalar_mul(
                        vsb[:, c, :], pvps[:, c, 0:D], rcp[:, c:c + 1])
                (nc.sync if (t % 2 == 0 or _NO_ACT_RING)
                 else nc.scalar).dma_start(
                    vout_ext[p, t * 512:(t + 1) * 512, :].rearrange(
                        "(c pp) d -> pp c d", pp=128),
                    vsb[:])
                return rcp

            _rings = (nc.sync, nc.scalar, nc.gpsimd)

            def series_dma(idx, dst, srcap):
                if _SKIP_SERIES_DMA:
                    return
                if _SERIES_DMA_SPLIT == 0:
                    nc.sync.dma_start(dst, srcap)
                elif _SERIES_DMA_SPLIT == 1:
                    (nc.sync, nc.scalar)[idx % 2].dma_start(dst, srcap)
                elif _NO_ACT_RING:
                    (nc.sync, nc.gpsimd)[idx % 2].dma_start(dst, srcap)
                elif not _SERIES_SPLIT_HALVES:
                    _rings[idx % 3].dma_start(dst, srcap)
                else:
                    _rings[(2 * idx) % 3].dma_start(dst[0:64], srcap[0:64])
                    _rings[(2 * idx + 1) % 3].dma_start(
                        dst[64:128], srcap[64:128])

            def pass2_tile(p, t, qt, kt, rcp, is_tail):
                if _SKIP_PASS2:
                    return
                for c in range(4):
                    C = 4 * t + c
                    W = (C + 1) * 128
                    row = rowp.tile([128, 2048], F32, tag="row")
                    tail_chunk = is_tail and c == 3
                    base = 0
                    while base < W:
                        w = min(512, W - base)
                        psr = ps2.tile([128, 512], F32, tag="s2")
                        nc.tensor.matmul(
                            psr[:, 0:w],
                            lhsT=qt[:, C * 128:(C + 1) * 128],
                            rhs=kt[:, base:base + w],
                            start=True, stop=True)
                        if base + w == W:
                            nc.vector.tensor_add(
                                psr[:, w - 128:w], psr[:, w - 128:w],
                                mask_d2[:])
                        nc.scalar.activation(
                            row[:, base:base + w], psr[:, 0:w],
                            AF.Exp, scale=SCALE)
                        if tail_chunk:
                            # kernel tail: normalize + store per instalment on
                            # DVE so the last DMA starts as early as possible
                            nc.vector.tensor_scalar_mul(
                                row[:, base:base + w], row[:, base:base + w],
                                rcp[:, c:c + 1])
                            series_dma(
                                base // 512,
                                ser_ext[p, C * 128:(C + 1) * 128,
                                        base:base + w],
                                row[:, base:base + w])
                        base += w
                    if not tail_chunk:
                        # normalize rows by 1/denom
                        if _SCALE_ON_DVE == "mix":
                            eng = nc.vector if (4 * t + c) % 2 else nc.gpsimd
                        else:
                            eng = nc.vector if _SCALE_ON_DVE else nc.gpsimd
                        eng.tensor_scalar_mul(
                            row[:, 0:W], row[:, 0:W], rcp[:, c:c + 1])
                        series_dma(
                            4 * t + c,
                            ser_ext[p, C * 128:(C + 1) * 128, 0:W],
                            row[:, 0:W])

            # ---- software pipeline over the 16 (pair, tile) units:
            # pass-1 of unit i+1 is emitted before pass-2 of unit i, so the
            # in-order ACT stream always has a segment whose inputs are ready.
            def emit_body():
                units = [(p, t) for p in range(PPC) for t in range(NT)]
                pair_bufs = {}
                qn0, kn0, va0 = load_pair(0)
                qt0, kt0 = transpose_pair(qn0, kn0)
                pair_bufs[0] = (qt0, kt0, va0)

                def unit_pass1(i):
                    p, t = units[i]
                    qt, kt, va = pair_bufs[p]
                    pv = pass1_tile(t, qt, kt, va)
                    if t == 1 and p + 1 < PPC:
                        qn2, kn2, va2 = load_pair(p + 1)
                        qt2, kt2 = transpose_pair(qn2, kn2)
                        pair_bufs[p + 1] = (qt2, kt2, va2)
                    return pv

                pv = unit_pass1(0)
                for i, (p, t) in enumerate(units):
                    rcp = denoms_and_vout(p, t, pv)
                    if i + 1 < len(units):
                        pv = unit_pass1(i + 1)
                    qt, kt, _ = pair_bufs[p]
                    pass2_tile(p, t, qt, kt, rcp,
                               is_tail=(i == len(units) - 1))
                    if t == NT - 1:
                        del pair_bufs[p]

            if loop_n is None:
                emit_body()
            else:
                with tc.For_i(0, loop_n, 1):
                    emit_body()

    nc.finalize()
    return nc


def kernel(queries, keys, values, sigma=None, **_unused):
    if "nc" not in _CACHE:
        _CACHE["nc"] = _build_nc()
    nc = _CACHE["nc"]

    q = np.ascontiguousarray(
        np.asarray(queries, np.float32).transpose(0, 2, 1, 3).reshape(PAIRS, L, E))
    k = np.ascontiguousarray(
        np.asarray(keys, np.float32).transpose(0, 2, 1, 3).reshape(PAIRS, L, E))
    v = np.ascontiguousarray(
        np.asarray(values, np.float32).transpose(0, 2, 1, 3).reshape(PAIRS, L, D))

    in_maps = [
        {"q": q[i * PPC:(i + 1) * PPC],
         "k": k[i * PPC:(i + 1) * PPC],
         "v": v[i * PPC:(i + 1) * PPC]}
        for i in range(NCORES)
    ]
    res = run_bass_kernel_spmd(nc, in_maps, list(range(NCORES)))

    vout = np.concatenate([res.results[i]["vout"] for i in range(NCORES)], 0)
    vout = vout.reshape(B, H, L, D).transpose(0, 2, 1, 3)
    ser = np.concatenate([res.results[i]["series"] for i in range(NCORES)], 0)
    ser = ser.reshape(B, H, L, L)
    return np.ascontiguousarray(vout), ser
